# revision 58
# baseline (speedup 1.0000x reference)
"""Windowed attention + dynamic relative position bias on 8 NeuronCores.

Shapes: q,k,v [B=128, H=8, N=256, D=32] f32; pos-MLP width P=16; h=w=16.
Sharding: head-parallel - core c computes head c for all 128 batch windows;
the per-core head is selected purely by the w3 column passed to that core
(program is SPMD-identical).

v2 design (Activation-engine bound; everything else hidden under it):
  - q/k are host-pretransposed: SBUF tiles ARE the qt/kt packs
    [(bi 4, d 32) partition, (g 16, hh 2, n 256) free] - no PE transposes,
    no PSUM evacuation copies.
  - q/k/v loaded in 5 big SWDGE chunks (Pool dispatch ~1us each).
  - QK: row-packed K=32 bf16 matmuls -> S^T [m 128, (j 2, mb 2, n 256)]
    PSUM tiles (2 per half-group); exp on ScalarE -> E bf16 (only Exp/Ln
    are ever used on ScalarE -> single activation-table load).
  - bias: exp(s+b) = exp(s)*exp(b); expb built once via the Toeplitz
    gather (posT -> DRAM -> 16 strided DMAs, half SWDGE half HWDGE ->
    J-matmul un-reversal -> exp); applied as ONE 4D DVE multiply per E
    tile (bf16 all-SBUF, 2x/4x DVE mode).
  - PV with ones-augmented V -> O_ps [128 n, (j 8, e 33)] PSUM, col 32 = Z;
    normalize fused into the PSUM evacuation (reciprocal + broadcast mul).
  - LN rsqrt on DVE (bitcast magic + 2 Newton steps) -> ScalarE only ever
    runs Exp -> exactly one activation-table load, no thrash.
  - pipeline: PV emitted 2 half-groups behind QK/exp so its deps are
    resolved before the PE sequencer reaches it (4-deep wait queue).
"""

import os
import numpy as np

B, H, N, D = 128, 8, 256, 32
P = 16
NCORES = 8
SCALE = float(1.0 / np.sqrt(D))
NGROUPS = 16
NPRE = int(os.environ.get("K_NPRE", "6"))          # prefix groups before expb
DRAIN = int(os.environ.get("K_DRAIN", "2"))        # extra PVs per new half-group
CHUNKS = [(0, 1), (2, 3), (4, 7), (8, 11), (12, 15)]
CHUNK_OF_GROUP = [0, 0, 1, 1, 2, 2, 2, 2, 3, 3, 3, 3, 4, 4, 4, 4]

# big const tiles, split so the stage-0/1 blocks land first (f32)
_CBA = {"bprojt": 0, "g1t": 128, "lb1t": 256, "linb1t": 384, "w1": 512}
_CBB = {"w2": 0, "ident": 128, "g2t": 256, "lb2t": 384, "linb2t": 512,
        "g3t": 640, "lb3t": 768, "w3c": 896}
CONSTWA = 640
CONSTWB = 904

_BUILD_CACHE = {}


def _build():
    if "nc" in _BUILD_CACHE:
        return _BUILD_CACHE["nc"]
    import concourse.bacc as bacc
    import concourse.mybir as mybir
    from concourse.tile import TileContext
    from bass_rust import AP

    F32 = mybir.dt.float32
    F32R = mybir.dt.float32r
    BF16 = mybir.dt.bfloat16
    AF = mybir.ActivationFunctionType
    AX = mybir.AxisListType
    ALU = mybir.AluOpType

    nc = bacc.Bacc("TRN2", target_bir_lowering=False, debug=False,
                   num_devices=NCORES)

    # host-prearranged layouts (see build_in_maps):
    # qd [128 p=(bi,d), (g 16, hh 2, n 256)]
    # kd [128 p=(bi,d), (g 16, hh 2, mb 2, m 128)]
    # vd [128 p=m, (b 128, c 2, e 33)] (e==32 -> 1.0)
    qd = nc.dram_tensor("qd", [128, 8192], F32, kind="ExternalInput")
    kd = nc.dram_tensor("kd", [128, 8192], F32, kind="ExternalInput")
    vd = nc.dram_tensor("vd", [128, 8448], F32, kind="ExternalInput")
    biasesT_d = nc.dram_tensor("biasesT", [2, 1024], F32, kind="ExternalInput")
    wproj_d = nc.dram_tensor("wproj", [2, P], F32, kind="ExternalInput")
    b3c_d = nc.dram_tensor("b3c", [8, 1], F32, kind="ExternalInput")
    jmat_d = nc.dram_tensor("jmat", [128, 128], F32, kind="ExternalInput")
    cbiga_d = nc.dram_tensor("cbiga", [128, CONSTWA], F32,
                             kind="ExternalInput")
    cbigb_d = nc.dram_tensor("cbigb", [128, CONSTWB], F32,
                             kind="ExternalInput")
    U32 = mybir.dt.uint32
    # group-0 of k, host-converted to bf16 and packed in uint32 pairs so a
    # cast-free HWDGE DMA can race the SWDGE q load at startup
    k0p_d = nc.dram_tensor("k0p", [128, 256], U32, kind="ExternalInput")

    posd = nc.dram_tensor("posd", [1, 1024], F32R, kind="Internal")
    out_d = nc.dram_tensor("out", [128, 8192], F32, kind="ExternalOutput")
    I32 = mybir.dt.int32

    with TileContext(nc) as tc:
        with (
            tc.tile_pool(name="const", bufs=1) as constp,
            tc.tile_pool(name="vpool", bufs=1) as vpool,
            tc.tile_pool(name="mlp", bufs=2) as mlpp,
            tc.tile_pool(name="epool", bufs=int(os.environ.get("K_EP", "28"))) as epool,
            tc.tile_pool(name="outp", bufs=int(os.environ.get("K_OUT", "8"))) as outp,
            tc.tile_pool(name="spsum", bufs=int(os.environ.get("K_SB", "3")), space="PSUM") as spsum,
            tc.tile_pool(name="auxpsum", bufs=int(os.environ.get("K_AB", "2")), space="PSUM") as auxpsum,
        ):
            # ---- full-size q/k/v SBUF tiles; chunked loads emitted lazily
            q_all = vpool.tile([128, 8192], BF16)
            k_all = vpool.tile([128, 8192], BF16)
            v_all = vpool.tile([128, 8448], BF16)

            chunk_loaded = [False] * len(CHUNKS)

            def emit_chunk(ci):
                g0, g1 = CHUNKS[ci]
                ng = g1 - g0 + 1
                qk0 = g0
                if ci == 0:
                    qk0 = 1      # group 0 of q/k loaded via the fast path
                for t_all, td in ((q_all, qd), (k_all, kd)):
                    nc.gpsimd.dma_start(
                        t_all[:, 512 * qk0:512 * (g1 + 1)],
                        AP(td, 512 * qk0,
                           [[8192, 128], [1, 512 * (g1 - qk0 + 1)]]))
                nc.gpsimd.dma_start(
                    v_all[:, 528 * g0:528 * (g1 + 1)],
                    AP(vd, 528 * g0, [[8448, 128], [1, 528 * ng]]))

            def ensure_chunk(ci):
                if not chunk_loaded[ci]:
                    chunk_loaded[ci] = True
                    emit_chunk(ci)

            # fast path for the very first QK: k (HWDGE, pre-packed bf16)
            # and q (SWDGE) go through different dispatchers concurrently;
            # the MLP consts follow right behind (transfers pipeline behind
            # the HWDGE generation slots, so cbiga lands only ~0.6us later)
            nc.sync.dma_start(k_all[:, 0:512].bitcast(U32), k0p_d[:, :])
            nc.gpsimd.dma_start(q_all[:, 0:512],
                                AP(qd, 0, [[8192, 128], [1, 512]]))

            biasesT = constp.tile([2, 1024], F32)
            nc.sync.dma_start(biasesT[:, :], biasesT_d[:, :])
            wproj = constp.tile([2, P], F32)
            nc.sync.dma_start(wproj[:, :], wproj_d[:, :])
            cbiga = constp.tile([128, CONSTWA], F32)
            nc.sync.dma_start(cbiga[:, :], cbiga_d[:, :])
            cbigb = constp.tile([128, CONSTWB], F32)
            nc.sync.dma_start(cbigb[:, :], cbigb_d[:, :])
            b3c = constp.tile([8, 1], F32)
            nc.sync.dma_start(b3c[:, :], b3c_d[:, :])
            jmat_f = constp.tile([128, 128], F32)
            nc.sync.dma_start(jmat_f[:, :], jmat_d[:, :])
            jmat_r = constp.tile([128, 128], F32R)
            nc.vector.tensor_copy(jmat_r[:, :], jmat_f[:, :])
            magic_t = constp.tile([128, 8], I32)
            nc.vector.memset(magic_t[:, :], 0x5F3759DF)

            ensure_chunk(0)
            ensure_chunk(1)

            def cb(nm):
                if nm in _CBA:
                    t, o = cbiga, _CBA[nm]
                else:
                    t, o = cbigb, _CBB[nm]
                w = 8 if nm == "w3c" else 128
                return t[:, o:o + w]

            mlp_env = {}

            def _mlp_layer(x_sb, g_t, beta_t, w_t, linb_t, last=False):
                x3 = x_sb[:, :].rearrange("p (j f) -> p j f", f=16)
                mz = mlpp.tile([128, 8], F32, tag="mz")
                nc.vector.tensor_reduce(mz[:, :], x3, AX.X, ALU.add)
                xc = mlpp.tile([128, 128], F32, tag="xc")
                xc3 = xc[:, :].rearrange("p (j f) -> p j f", f=16)
                # xc' = mz/16 - x  (negated; g tiles are host-negated)
                nc.vector.scalar_tensor_tensor(
                    xc3, mz[:, :].unsqueeze(2).broadcast_to((128, 8, 16)),
                    1.0 / 16.0, x3, ALU.mult, ALU.subtract)
                sq = mlpp.tile([128, 128], F32, tag="sq")
                sq3 = sq[:, :].rearrange("p (j f) -> p j f", f=16)
                nc.vector.tensor_mul(sq3, xc3, xc3)
                vz = mlpp.tile([128, 8], F32, tag="vz")
                nc.vector.tensor_reduce(vz[:, :], sq3, AX.X, ALU.add)
                # rsqrt(v/16 + eps) fully on DVE (bit-magic + 2 Newton
                # steps) so ScalarE only ever runs Exp (one act table).
                w = mlpp.tile([128, 8], F32, tag="w")
                nc.vector.tensor_scalar(w[:, :], vz[:, :], 1.0 / 16.0, 1e-5,
                                        ALU.mult, ALU.add)
                sh = mlpp.tile([128, 8], I32, tag="sh")
                nc.vector.tensor_single_scalar(sh[:, :],
                                               w[:, :].bitcast(I32), 1,
                                               ALU.arith_shift_right)
                yi = mlpp.tile([128, 8], I32, tag="yi")
                nc.vector.tensor_sub(yi[:, :], magic_t[:, :], sh[:, :])
                y0 = yi[:, :].bitcast(F32)
                rz = None
                for it in range(int(os.environ.get("K_NEWTON", "1"))):
                    t = mlpp.tile([128, 8], F32, tag=f"nt{it}")
                    nc.vector.tensor_mul(t[:, :], w[:, :], y0)
                    nc.vector.tensor_mul(t[:, :], t[:, :], y0)
                    nc.vector.tensor_scalar(t[:, :], t[:, :], -0.5, 1.5,
                                            ALU.mult, ALU.add)
                    y1 = mlpp.tile([128, 8], F32, tag=f"ny{it}")
                    nc.vector.tensor_mul(y1[:, :], y0, t[:, :])
                    y0 = y1[:, :]
                    rz = y1
                xn = mlpp.tile([128, 128], F32, tag="xn")
                xn3 = xn[:, :].rearrange("p (j f) -> p j f", f=16)
                nc.vector.tensor_mul(
                    xn3, xc3, rz[:, :].unsqueeze(2).broadcast_to((128, 8, 16)))
                y = mlpp.tile([128, 128], F32, tag="y")
                nc.vector.tensor_mul(y[:, :], xn[:, :], g_t[:, :])
                # +beta folded into the transpose (PSUM accumulate of the
                # column-broadcast betaT const); relu folded into the evac
                pt = auxpsum.tile([128, 512], F32, tag="aux2")
                nc.tensor.matmul(pt[:, :128], y[:, :], cb("ident"),
                                 is_transpose=True, start=True, stop=False)
                nc.tensor.matmul(pt[:, :128], cb("ident"), beta_t,
                                 start=False, stop=True)
                yT = mlpp.tile([128, 128], F32, tag="yT")
                nc.vector.tensor_scalar_max(yT[:, :], pt[:, :128], 0.0)
                if last:
                    return yT
                px = auxpsum.tile([128, 512], F32, tag="aux2")
                nc.tensor.matmul(px[:, :128], yT[:, :], w_t)
                xnext = mlpp.tile([128, 128], F32, tag="xnext")
                nc.vector.tensor_add(xnext[:, :], px[:, :128], linb_t)
                return xnext

            def emit_mlp_stage(stage):
                """0=x0, 1..3=LN layers, 4=pos->DRAM->gather (sets btrev)."""
                env = mlp_env
                if stage == 0:
                    px0 = auxpsum.tile([128, 512], F32, tag="aux2")
                    for j in range(8):
                        nc.tensor.matmul(px0[:, 16 * j:16 * j + 16],
                                         biasesT[:, 128 * j:128 * j + 128],
                                         wproj[:, :])
                    x0 = mlpp.tile([128, 128], F32, tag="x0")
                    nc.vector.tensor_add(x0[:, :], px0[:, :128], cb("bprojt"))
                    env["x0"] = x0
                    return
                if stage == 1:
                    env["x1"] = _mlp_layer(env["x0"], cb("g1t"), cb("lb1t"),
                                           cb("w1"), cb("linb1t"))
                    return
                if stage == 2:
                    env["x2"] = _mlp_layer(env["x1"], cb("g2t"), cb("lb2t"),
                                           cb("w2"), cb("linb2t"))
                    return
                if stage == 3:
                    env["y3T"] = _mlp_layer(env["x2"], cb("g3t"), cb("lb3t"),
                                            None, None, last=True)
                    return
                # stage 4: posT -> DRAM -> Toeplitz gather (reversed m)
                pos_ps = auxpsum.tile([128, 512], F32, tag="aux2")
                nc.tensor.matmul(pos_ps[0:8, :128], cb("w3c"), env["y3T"][:, :])
                pos_sb = constp.tile([8, 128], F32R)
                nc.vector.tensor_scalar_add(pos_sb[:, :], pos_ps[0:8, :128],
                                            b3c[:, 0:1])
                nc.sync.dma_start(AP(posd, 0, [[128, 8], [1, 128]]),
                                  pos_sb[:, :])
                btrev = []
                gi = 0
                for mbp in range(2):
                    bt = constp.tile([128, 256], F32R, tag=f"btrev{mbp}")
                    btrev.append(bt)
                    for a in range(8):
                        src = AP(posd, 31 * (8 * mbp + a),
                                 [[1, 16], [31, 16], [1, 16]])
                        dst = bt[16 * a:16 * a + 16, :].rearrange(
                            "b (c e) -> b c e", e=16)
                        # 10 on HWDGE (625ns each) / 6 on Pool (1081ns
                        # each) so both dispatchers finish together
                        if gi % 8 < 5:
                            nc.sync.dma_start(dst, src)
                        else:
                            nc.gpsimd.dma_start(dst, src)
                        gi += 1
                env["btrev"] = btrev

            def emit_expb():
                btrev = mlp_env["btrev"]
                expb = constp.tile([128, 512], BF16)
                for mb in range(2):
                    pe_ = auxpsum.tile([128, 512], F32, tag="aux2",
                                       name=f"pexpb{mb}")
                    nc.tensor.matmul(pe_[:, :256], jmat_r[:, :],
                                     btrev[1 - mb][:, :])
                    nc.scalar.activation(expb[:, 256 * mb:256 * mb + 256],
                                         pe_[:, :256], AF.Exp, scale=SCALE)
                return expb

            # --- main pipeline over 32 half-groups ---
            def emit_qk_exp(g, hh, pe_bias=False, split_exp=False):
                """QK matmuls + exp for half-group (g, hh) -> epair.

                pe_bias: accumulate sqrt(D)*bias into S via a J-matmul
                (needs btrev); exp then applies it, no DVE bias-mul needed.
                """
                ho = 512 * g + 256 * hh
                epair = []
                for half in range(2):
                    sp = spsum.tile([128, 1024], F32, tag="S",
                                    name=f"s{g}_{hh}_{half}")
                    for bi2 in range(2):
                        bi = 2 * half + bi2
                        fo = 512 * bi2
                        for mb in range(2):
                            out_ap = sp[:, fo + 256 * mb:fo + 256 * mb + 256]
                            nc.tensor.matmul(
                                out_ap,
                                k_all[32 * bi:32 * bi + 32,
                                      ho + 128 * mb:ho + 128 * mb + 128],
                                q_all[32 * bi:32 * bi + 32, ho:ho + 256],
                                tile_position=(32 * bi, 0),
                                start=True, stop=not pe_bias)
                            if pe_bias:
                                nc.tensor.matmul(
                                    out_ap, jmat_r[:, :],
                                    mlp_env["btrev"][1 - mb][:, :],
                                    tile_position=(0, 0),
                                    start=False, stop=True)
                    e = epool.tile([128, 1024], BF16, tag="E",
                                   name=f"e{g}_{hh}_{half}")
                    if split_exp:
                        # two 512-wide exps so the first can start as soon
                        # as the bi2=0 QK matmuls land (pipeline startup)
                        nc.scalar.activation(e[:, :512], sp[:, :512],
                                             AF.Exp, scale=SCALE)
                        nc.scalar.activation(e[:, 512:], sp[:, 512:],
                                             AF.Exp, scale=SCALE)
                    else:
                        nc.scalar.activation(e[:, :], sp[:, :], AF.Exp,
                                             scale=SCALE)
                    epair.append(e)
                return epair

            def emit_emul(expb, epair):
                for e in epair:
                    e4 = e[:, :].rearrange("p (j mb n) -> p j mb n",
                                           mb=2, n=256)
                    nc.vector.tensor_mul(
                        e4, e4,
                        expb[:, :].rearrange("p (mb n) -> p mb n", n=256)
                        .unsqueeze(1).broadcast_to((128, 2, 2, 256)))

            def emit_pv_out(g, hh, epair, split=False):
                o_ps = auxpsum.tile([128, 264], F32, tag="aux2",
                                    name=f"ops{g}_{hh}")

                def pv(bis):
                    for bi in bis:
                        e = epair[bi // 2]
                        fo = 512 * (bi % 2)
                        vb = 66 * (8 * g + 4 * hh + bi)
                        for nb in range(2):
                            j = 2 * bi + nb
                            for c in range(2):
                                nc.tensor.matmul(
                                    o_ps[:, 33 * j:33 * j + 33],
                                    e[:, fo + 256 * c + 128 * nb:
                                      fo + 256 * c + 128 * nb + 128],
                                    v_all[:, vb + 33 * c:vb + 33 * c + 33],
                                    start=(c == 0), stop=(c == 1))

                def norm_out(j0, nj):
                    rz = outp.tile([128, 8], F32, tag="rz",
                                   name=f"rz{g}_{hh}_{j0}")
                    o3 = o_ps[:, 33 * j0:33 * (j0 + nj)].rearrange(
                        "p (j e) -> p j e", e=33)
                    nc.vector.reciprocal(rz[:, :nj], o3[:, :, 32:33])
                    osb = outp.tile([128, 256], F32, tag="osb",
                                    name=f"osb{g}_{hh}_{j0}")
                    nc.vector.tensor_mul(
                        osb[:, :32 * nj].rearrange("p (j e) -> p j e", e=32),
                        o3[:, :, :32],
                        rz[:, :nj].unsqueeze(2).broadcast_to((128, nj, 32)))
                    nc.sync.dma_start(
                        AP(out_d, 256 * (2 * g + hh) + 32 * j0,
                           [[8192, 128], [1, 32 * nj]]),
                        osb[:, :32 * nj])

                if split:
                    # each j column-group of O (incl. its Z) is complete
                    # after its own matmuls: half the PV/norm/out overlaps
                    # the other half's exp at the kernel tail
                    pv((0, 1))
                    norm_out(0, 4)
                    pv((2, 3))
                    norm_out(4, 4)
                else:
                    pv(range(4))
                    norm_out(0, 8)

            # ---- schedule ----
            # unmul: exp'd, bias-mul not yet emitted; unpv: mul'd, PV not
            # yet emitted. Steady state: mul lags 1 half-group, PV lags 2,
            # and PVs are emitted BEFORE the new QK so their deps are
            # already satisfied when the PE sequencer reaches them.
            unmul = []
            unpv = []

            # Prefix: interleave the MLP's few PE ops between prefix QKs so
            # each lands in the in-order PE stream AFTER its DVE input is
            # ready but long BEFORE the prefix drains -> posd/gathers/expb
            # complete under the prefix exps.  MLP stage s goes after
            # half-group s+1 (first QK feeds Act ASAP, stage0 needs only
            # the tiny biasesT/wproj DMAs).
            stage_after = {1: 1, 2: 2, 3: 3, 4: 4}  # halfgroup -> mlp stage
            hg = 0
            emit_mlp_stage(0)
            for g in range(NPRE):
                ensure_chunk(CHUNK_OF_GROUP[min(g + 2, NGROUPS - 1)])
                for hh in range(2):
                    unmul.append((g, hh,
                                  emit_qk_exp(g, hh, split_exp=(g == 0)),
                                  False))
                    hg += 1
                    st = stage_after.get(hg)
                    if st == 4:
                        # keep Pool's queue ahead of the gathers
                        ensure_chunk(CHUNK_OF_GROUP[4])
                    if st is not None:
                        emit_mlp_stage(st)
            expb = emit_expb()

            FOLD = int(os.environ.get("K_FOLD", "2"))
            for g in range(NPRE, NGROUPS):
                ensure_chunk(CHUNK_OF_GROUP[min(g + 2, NGROUPS - 1)])
                for hh in range(2):
                    endgame = g >= NGROUPS - 3
                    drain = DRAIN if not endgame else 4
                    if endgame:
                        # lag 1 at the end: deps are long-satisfied and a
                        # short PE wait no longer starves anything
                        for _ in range(drain):
                            if unmul:
                                g_, hh_, ep_, folded = unmul.pop(0)
                                if not folded:
                                    emit_emul(expb, ep_)
                                unpv.append((g_, hh_, ep_))
                    for _ in range(drain):
                        if unpv:
                            emit_pv_out(*unpv.pop(0))
                    if not endgame:
                        for _ in range(drain):
                            if unmul:
                                g_, hh_, ep_, folded = unmul.pop(0)
                                if not folded:
                                    emit_emul(expb, ep_)
                                unpv.append((g_, hh_, ep_))
                    # g==NPRE stays unfolded: its exps need no btrev, so
                    # they bridge the gather-latency window right after
                    # the prefix; g==NPRE+1 folds both half-groups to
                    # compensate, keeping the total DVE mul count equal
                    fold = ((FOLD == 1)
                            or (FOLD == 2 and (2 * g + hh) % 2 == 0)
                            or g == NPRE + 1
                            or g >= NGROUPS - 2) and g != NPRE
                    unmul.append((g, hh, emit_qk_exp(g, hh, pe_bias=fold),
                                  fold))
            for g_, hh_, ep_, folded in unmul:
                if not folded:
                    emit_emul(expb, ep_)
                unpv.append((g_, hh_, ep_))
            # newest first: the critical exp->PV->norm->DMA chain of the
            # final half-group must not queue behind older pending norms
            for item in reversed(unpv):
                emit_pv_out(*item)

    nc.compile()
    _BUILD_CACHE["nc"] = nc
    return nc


def _host_constants():
    hh, ww = 16, 16
    bh, bw = np.meshgrid(np.arange(1 - hh, hh), np.arange(1 - ww, ww),
                         indexing="ij")
    biases = np.stack([bh, bw], -1).reshape(-1, 2).astype(np.float32)
    biasesT = np.zeros((2, 1024), np.float32)
    biasesT[:, :961] = biases.T
    return biasesT


def _blk8(w16):
    cout = w16.shape[1]
    blk = np.zeros((128, 8 * cout), np.float32)
    for j in range(8):
        blk[16 * j:16 * j + 16, cout * j:cout * j + cout] = w16
    return np.ascontiguousarray(blk)


def _tile16(vec):
    return np.ascontiguousarray(
        np.tile(np.asarray(vec, np.float32), (128, 8)))


def _tile16T(vec):
    # column-broadcast (transposed-space) tiling: out[16j+f, r] = vec[f]
    col = np.tile(np.asarray(vec, np.float32), 8)[:, None]
    return np.ascontiguousarray(np.tile(col, (1, 128)))


def build_in_maps(inputs):
    q = np.asarray(inputs["q"], np.float32)
    k = np.asarray(inputs["k"], np.float32)
    v = np.asarray(inputs["v"], np.float32)
    hh = int(np.asarray(inputs["h"]))
    ww = int(np.asarray(inputs["w"]))
    assert hh == 16 and ww == 16, (hh, ww)
    f32 = lambda name: np.asarray(inputs[name], np.float32)
    w3 = f32("w3")
    b3 = f32("b3")
    sqrtD = np.float32(np.sqrt(D))

    cblk = {
        "w1": _blk8(f32("w1")), "w2": _blk8(f32("w2")),
        "ident": np.eye(128, dtype=np.float32),
        "bprojt": _tile16(f32("b_proj")),
        "g1t": -_tile16(f32("ln1_g")), "lb1t": _tile16T(f32("ln1_b")),
        "linb1t": _tile16(f32("b1")),
        "g2t": -_tile16(f32("ln2_g")), "lb2t": _tile16T(f32("ln2_b")),
        "linb2t": _tile16(f32("b2")),
        "g3t": -_tile16(f32("ln3_g")), "lb3t": _tile16T(f32("ln3_b")),
    }
    shared = {
        "biasesT": _host_constants(),
        "wproj": f32("w_proj"),
        "jmat": np.eye(128, dtype=np.float32)[::-1].copy(),
    }

    def q_layout(x):
        # [128 w, 256 n, 32 d] -> [128 p=(bi,d), (g, hh, n)]
        x5 = x.reshape(16, 2, 4, 256, 32)           # g hh bi n d
        return np.ascontiguousarray(
            x5.transpose(2, 4, 0, 1, 3).reshape(128, 8192))

    def k_layout(x):
        # [128 w, 256 m, 32 d] -> [128 p=(bi,d), (g, hh, mb, m)]
        x6 = x.reshape(16, 2, 4, 2, 128, 32)        # g hh bi mb m d
        return np.ascontiguousarray(
            x6.transpose(2, 5, 0, 1, 3, 4).reshape(128, 8192))

    def pack_bf16_pairs(x):
        # f32 [128, 512] -> bf16 (RNE) bit-packed into uint32 [128, 256]
        u = np.ascontiguousarray(x).view(np.uint32)
        r = ((u + 0x7FFF + ((u >> 16) & 1)) >> 16).astype(np.uint32)
        return np.ascontiguousarray(r[:, 0::2] | (r[:, 1::2] << 16))

    def v_layout(x):
        # [128 p=m, (b 128, c 2, e 33)]; e==32 -> 1.0
        v4 = x.reshape(128, 2, 128, 32)             # b c p e
        out = np.ones((128, 128, 2, 33), np.float32)
        out[:, :, :, :32] = v4.transpose(2, 0, 1, 3)
        return np.ascontiguousarray(out.reshape(128, 8448))

    in_maps = []
    for c in range(NCORES):
        cbiga = np.empty((128, CONSTWA), np.float32)
        for nm, off in _CBA.items():
            cbiga[:, off:off + 128] = cblk[nm]
        cbigb = np.empty((128, CONSTWB), np.float32)
        for nm, off in _CBB.items():
            if nm == "w3c":
                cbigb[:, off:off + 8] = _blk8(w3[:, c:c + 1] * sqrtD)
            else:
                cbigb[:, off:off + 128] = cblk[nm]
        m = dict(shared)
        m["cbiga"] = np.ascontiguousarray(cbiga)
        m["cbigb"] = np.ascontiguousarray(cbigb)
        m["qd"] = q_layout(q[:, c])
        m["kd"] = k_layout(k[:, c])
        m["k0p"] = pack_bf16_pairs(m["kd"][:, 0:512])
        m["vd"] = v_layout(v[:, c])
        m["b3c"] = np.full((8, 1), b3[c], np.float32) * sqrtD
        in_maps.append(m)
    return in_maps


def unshard_out(raw):
    # raw [128 p, (g 16, hh 2, bi 4, nb 2, e 32)] -> [B, N, D]
    r6 = raw.reshape(128, 16, 2, 4, 2, 32)          # p g hh bi nb e
    return np.ascontiguousarray(
        r6.transpose(1, 2, 3, 4, 0, 5).reshape(128, 256, 32))


def kernel(**inputs):
    from concourse.bass_utils import run_bass_kernel_spmd

    nc = _build()
    in_maps = build_in_maps(inputs)
    res = run_bass_kernel_spmd(nc, in_maps, core_ids=list(range(NCORES)))
    out = np.empty((B, H, N, D), np.float32)
    for c in range(NCORES):
        out[:, c] = unshard_out(res.results[c]["out"])
    return out


# revision 59
# speedup vs baseline: 1.0190x; 1.0190x over previous
"""Windowed attention + dynamic relative position bias on 8 NeuronCores.

Shapes: q,k,v [B=128, H=8, N=256, D=32] f32; pos-MLP width P=16; h=w=16.
Sharding: head-parallel - core c computes head c for all 128 batch windows;
the per-core head is selected purely by the w3 column passed to that core
(program is SPMD-identical).

Design (Activation-engine bound, ~66us of exp; everything else hides
under it; modeled span ~79us vs 119us for the previous version):
  - q/k are host-pretransposed: SBUF tiles ARE the qt/kt packs
    [(bi 4, d 32) partition, (g 16, hh 2, n 256) free] - no PE transposes,
    no PSUM evacuation copies.
  - q/k/v loaded in 5 big SWDGE chunks (Pool dispatch ~1us each); the
    very first k slice goes as host-packed bf16 bits over HWDGE so it
    races the SWDGE q load (first exp at ~4.4us).
  - QK: row-packed K=32 bf16 matmuls -> S^T [m 128, (j 2, mb 2, n 256)]
    PSUM tiles (2 per half-group, triple buffered); exp on ScalarE ->
    E bf16. ScalarE only ever runs Exp -> exactly one activation-table
    load (the LN rsqrt runs on DVE via bitcast magic + 1 Newton step).
  - bias: exp(s+b) = exp(s)*exp(b). expb is built once via the Toeplitz
    gather (posT -> DRAM -> 16 strided DMAs, 10 HWDGE + 6 SWDGE in
    parallel -> J-matmul un-reversal -> exp). For roughly half the
    half-groups (fold) the sqrt(D)*bias is accumulated into S by a
    J-matmul on the PE; for the rest it is a 4D DVE multiply on E -
    balancing PE vs DVE so neither exceeds the ScalarE budget. Group
    NPRE stays unfolded to bridge the gather-latency window.
  - MLP beta-add folded into the PE transpose (PSUM accumulate of a
    column-broadcast betaT const); relu folded into the evacuation max.
  - PV with ones-augmented V -> O_ps [128 n, (j 8, e 33)] PSUM, col 32 =
    Z; normalize fused into the PSUM evacuation (reciprocal + broadcast
    mul, deep osb pool so DMA completion never gates the next chain).
  - pipeline: engines execute in order, so the MLP's PE matmuls are
    interleaved between prefix QKs (deps ready when reached), PV runs
    ~2 half-groups behind QK/exp, and the prefix backlog drains at
    DRAIN extra PV/mul per new half-group.
"""

import os
import numpy as np

B, H, N, D = 128, 8, 256, 32
P = 16
NCORES = 8
SCALE = float(1.0 / np.sqrt(D))
NGROUPS = 16
NPRE = int(os.environ.get("K_NPRE", "6"))          # prefix groups before expb
DRAIN = int(os.environ.get("K_DRAIN", "2"))        # extra PVs per new half-group
CHUNKS = [(0, 1), (2, 3), (4, 7), (8, 11), (12, 15)]
CHUNK_OF_GROUP = [0, 0, 1, 1, 2, 2, 2, 2, 3, 3, 3, 3, 4, 4, 4, 4]

# big const tiles, split so the stage-0/1 blocks land first (f32)
_CBA = {"bprojt": 0, "g1t": 128, "lb1t": 256, "linb1t": 384, "w1": 512}
_CBB = {"w2": 0, "ident": 128, "g2t": 256, "lb2t": 384, "linb2t": 512,
        "g3t": 640, "lb3t": 768, "w3c": 896}
CONSTWA = 640
CONSTWB = 904

_BUILD_CACHE = {}


def _build():
    if "nc" in _BUILD_CACHE:
        return _BUILD_CACHE["nc"]
    import concourse.bacc as bacc
    import concourse.mybir as mybir
    from concourse.tile import TileContext
    from bass_rust import AP

    F32 = mybir.dt.float32
    F32R = mybir.dt.float32r
    BF16 = mybir.dt.bfloat16
    AF = mybir.ActivationFunctionType
    AX = mybir.AxisListType
    ALU = mybir.AluOpType

    nc = bacc.Bacc("TRN2", target_bir_lowering=False, debug=False,
                   num_devices=NCORES)

    # host-prearranged layouts (see build_in_maps):
    # qd [128 p=(bi,d), (g 16, hh 2, n 256)]
    # kd [128 p=(bi,d), (g 16, hh 2, mb 2, m 128)]
    # vd [128 p=m, (b 128, c 2, e 33)] (e==32 -> 1.0)
    qd = nc.dram_tensor("qd", [128, 8192], F32, kind="ExternalInput")
    kd = nc.dram_tensor("kd", [128, 8192], F32, kind="ExternalInput")
    vd = nc.dram_tensor("vd", [128, 8448], F32, kind="ExternalInput")
    biasesT_d = nc.dram_tensor("biasesT", [2, 1024], F32, kind="ExternalInput")
    wproj_d = nc.dram_tensor("wproj", [2, P], F32, kind="ExternalInput")
    b3c_d = nc.dram_tensor("b3c", [8, 1], F32, kind="ExternalInput")
    jmat_d = nc.dram_tensor("jmat", [128, 128], F32, kind="ExternalInput")
    cbiga_d = nc.dram_tensor("cbiga", [128, CONSTWA], F32,
                             kind="ExternalInput")
    cbigb_d = nc.dram_tensor("cbigb", [128, CONSTWB], F32,
                             kind="ExternalInput")
    U32 = mybir.dt.uint32
    # group-0 of k, host-converted to bf16 and packed in uint32 pairs so a
    # cast-free HWDGE DMA can race the SWDGE q load at startup
    k0p_d = nc.dram_tensor("k0p", [128, 256], U32, kind="ExternalInput")

    posd = nc.dram_tensor("posd", [1, 1024], F32R, kind="Internal")
    out_d = nc.dram_tensor("out", [128, 8192], F32, kind="ExternalOutput")
    I32 = mybir.dt.int32

    with TileContext(nc) as tc:
        with (
            tc.tile_pool(name="const", bufs=1) as constp,
            tc.tile_pool(name="vpool", bufs=1) as vpool,
            tc.tile_pool(name="mlp", bufs=2) as mlpp,
            tc.tile_pool(name="epool", bufs=int(os.environ.get("K_EP", "28"))) as epool,
            tc.tile_pool(name="outp", bufs=int(os.environ.get("K_OUT", "8"))) as outp,
            tc.tile_pool(name="spsum", bufs=int(os.environ.get("K_SB", "3")), space="PSUM") as spsum,
            tc.tile_pool(name="auxpsum", bufs=int(os.environ.get("K_AB", "2")), space="PSUM") as auxpsum,
        ):
            # ---- full-size q/k/v SBUF tiles; chunked loads emitted lazily
            q_all = vpool.tile([128, 8192], BF16)
            k_all = vpool.tile([128, 8192], BF16)
            v_all = vpool.tile([128, 8448], BF16)

            chunk_loaded = [False] * len(CHUNKS)

            def emit_chunk(ci):
                g0, g1 = CHUNKS[ci]
                ng = g1 - g0 + 1
                qk0 = g0
                if ci == 0:
                    qk0 = 1      # group 0 of q/k loaded via the fast path
                for t_all, td in ((q_all, qd), (k_all, kd)):
                    nc.gpsimd.dma_start(
                        t_all[:, 512 * qk0:512 * (g1 + 1)],
                        AP(td, 512 * qk0,
                           [[8192, 128], [1, 512 * (g1 - qk0 + 1)]]))
                nc.gpsimd.dma_start(
                    v_all[:, 528 * g0:528 * (g1 + 1)],
                    AP(vd, 528 * g0, [[8448, 128], [1, 528 * ng]]))

            def ensure_chunk(ci):
                if not chunk_loaded[ci]:
                    chunk_loaded[ci] = True
                    emit_chunk(ci)

            # fast path for the very first QK: k (HWDGE, pre-packed bf16)
            # and q (SWDGE) go through different dispatchers concurrently;
            # the MLP consts follow right behind (transfers pipeline behind
            # the HWDGE generation slots, so cbiga lands only ~0.6us later)
            nc.sync.dma_start(k_all[:, 0:512].bitcast(U32), k0p_d[:, :])
            nc.gpsimd.dma_start(q_all[:, 0:512],
                                AP(qd, 0, [[8192, 128], [1, 512]]))

            biasesT = constp.tile([2, 1024], F32)
            nc.sync.dma_start(biasesT[:, :], biasesT_d[:, :])
            wproj = constp.tile([2, P], F32)
            nc.sync.dma_start(wproj[:, :], wproj_d[:, :])
            cbiga = constp.tile([128, CONSTWA], F32)
            nc.sync.dma_start(cbiga[:, :], cbiga_d[:, :])
            cbigb = constp.tile([128, CONSTWB], F32)
            nc.sync.dma_start(cbigb[:, :], cbigb_d[:, :])
            b3c = constp.tile([8, 1], F32)
            nc.sync.dma_start(b3c[:, :], b3c_d[:, :])
            jmat_f = constp.tile([128, 128], F32)
            nc.sync.dma_start(jmat_f[:, :], jmat_d[:, :])
            jmat_r = constp.tile([128, 128], F32R)
            nc.vector.tensor_copy(jmat_r[:, :], jmat_f[:, :])
            magic_t = constp.tile([128, 8], I32)
            nc.vector.memset(magic_t[:, :], 0x5F3759DF)

            ensure_chunk(0)
            ensure_chunk(1)

            def cb(nm):
                if nm in _CBA:
                    t, o = cbiga, _CBA[nm]
                else:
                    t, o = cbigb, _CBB[nm]
                w = 8 if nm == "w3c" else 128
                return t[:, o:o + w]

            mlp_env = {}

            def _mlp_layer(x_sb, g_t, beta_t, w_t, linb_t, last=False):
                x3 = x_sb[:, :].rearrange("p (j f) -> p j f", f=16)
                mz = mlpp.tile([128, 8], F32, tag="mz")
                nc.vector.tensor_reduce(mz[:, :], x3, AX.X, ALU.add)
                xc = mlpp.tile([128, 128], F32, tag="xc")
                xc3 = xc[:, :].rearrange("p (j f) -> p j f", f=16)
                # xc' = mz/16 - x  (negated; g tiles are host-negated)
                nc.vector.scalar_tensor_tensor(
                    xc3, mz[:, :].unsqueeze(2).broadcast_to((128, 8, 16)),
                    1.0 / 16.0, x3, ALU.mult, ALU.subtract)
                sq = mlpp.tile([128, 128], F32, tag="sq")
                sq3 = sq[:, :].rearrange("p (j f) -> p j f", f=16)
                nc.vector.tensor_mul(sq3, xc3, xc3)
                vz = mlpp.tile([128, 8], F32, tag="vz")
                nc.vector.tensor_reduce(vz[:, :], sq3, AX.X, ALU.add)
                # rsqrt(v/16 + eps) fully on DVE (bit-magic + 2 Newton
                # steps) so ScalarE only ever runs Exp (one act table).
                w = mlpp.tile([128, 8], F32, tag="w")
                nc.vector.tensor_scalar(w[:, :], vz[:, :], 1.0 / 16.0, 1e-5,
                                        ALU.mult, ALU.add)
                sh = mlpp.tile([128, 8], I32, tag="sh")
                nc.vector.tensor_single_scalar(sh[:, :],
                                               w[:, :].bitcast(I32), 1,
                                               ALU.arith_shift_right)
                yi = mlpp.tile([128, 8], I32, tag="yi")
                nc.vector.tensor_sub(yi[:, :], magic_t[:, :], sh[:, :])
                y0 = yi[:, :].bitcast(F32)
                rz = None
                for it in range(int(os.environ.get("K_NEWTON", "1"))):
                    t = mlpp.tile([128, 8], F32, tag=f"nt{it}")
                    nc.vector.tensor_mul(t[:, :], w[:, :], y0)
                    nc.vector.tensor_mul(t[:, :], t[:, :], y0)
                    nc.vector.tensor_scalar(t[:, :], t[:, :], -0.5, 1.5,
                                            ALU.mult, ALU.add)
                    y1 = mlpp.tile([128, 8], F32, tag=f"ny{it}")
                    nc.vector.tensor_mul(y1[:, :], y0, t[:, :])
                    y0 = y1[:, :]
                    rz = y1
                xn = mlpp.tile([128, 128], F32, tag="xn")
                xn3 = xn[:, :].rearrange("p (j f) -> p j f", f=16)
                nc.vector.tensor_mul(
                    xn3, xc3, rz[:, :].unsqueeze(2).broadcast_to((128, 8, 16)))
                y = mlpp.tile([128, 128], F32, tag="y")
                nc.vector.tensor_mul(y[:, :], xn[:, :], g_t[:, :])
                # +beta folded into the transpose (PSUM accumulate of the
                # column-broadcast betaT const); relu folded into the evac
                pt = auxpsum.tile([128, 512], F32, tag="aux2")
                nc.tensor.matmul(pt[:, :128], y[:, :], cb("ident"),
                                 is_transpose=True, start=True, stop=False)
                nc.tensor.matmul(pt[:, :128], cb("ident"), beta_t,
                                 start=False, stop=True)
                yT = mlpp.tile([128, 128], F32, tag="yT")
                nc.vector.tensor_scalar_max(yT[:, :], pt[:, :128], 0.0)
                if last:
                    return yT
                px = auxpsum.tile([128, 512], F32, tag="aux2")
                nc.tensor.matmul(px[:, :128], yT[:, :], w_t)
                xnext = mlpp.tile([128, 128], F32, tag="xnext")
                nc.vector.tensor_add(xnext[:, :], px[:, :128], linb_t)
                return xnext

            def emit_mlp_stage(stage):
                """0=x0, 1..3=LN layers, 4=pos->DRAM->gather (sets btrev)."""
                env = mlp_env
                if stage == 0:
                    px0 = auxpsum.tile([128, 512], F32, tag="aux2")
                    for j in range(8):
                        nc.tensor.matmul(px0[:, 16 * j:16 * j + 16],
                                         biasesT[:, 128 * j:128 * j + 128],
                                         wproj[:, :])
                    x0 = mlpp.tile([128, 128], F32, tag="x0")
                    nc.vector.tensor_add(x0[:, :], px0[:, :128], cb("bprojt"))
                    env["x0"] = x0
                    return
                if stage == 1:
                    env["x1"] = _mlp_layer(env["x0"], cb("g1t"), cb("lb1t"),
                                           cb("w1"), cb("linb1t"))
                    return
                if stage == 2:
                    env["x2"] = _mlp_layer(env["x1"], cb("g2t"), cb("lb2t"),
                                           cb("w2"), cb("linb2t"))
                    return
                if stage == 3:
                    env["y3T"] = _mlp_layer(env["x2"], cb("g3t"), cb("lb3t"),
                                            None, None, last=True)
                    return
                # stage 4: posT -> DRAM -> Toeplitz gather (reversed m)
                pos_ps = auxpsum.tile([128, 512], F32, tag="aux2")
                nc.tensor.matmul(pos_ps[0:8, :128], cb("w3c"), env["y3T"][:, :])
                pos_sb = constp.tile([8, 128], F32R)
                nc.vector.tensor_scalar_add(pos_sb[:, :], pos_ps[0:8, :128],
                                            b3c[:, 0:1])
                nc.sync.dma_start(AP(posd, 0, [[128, 8], [1, 128]]),
                                  pos_sb[:, :])
                btrev = []
                gi = 0
                for mbp in range(2):
                    bt = constp.tile([128, 256], F32R, tag=f"btrev{mbp}")
                    btrev.append(bt)
                    for a in range(8):
                        src = AP(posd, 31 * (8 * mbp + a),
                                 [[1, 16], [31, 16], [1, 16]])
                        dst = bt[16 * a:16 * a + 16, :].rearrange(
                            "b (c e) -> b c e", e=16)
                        # 10 on HWDGE (625ns each) / 6 on Pool (1081ns
                        # each) so both dispatchers finish together
                        if gi % 8 < 5:
                            nc.sync.dma_start(dst, src)
                        else:
                            nc.gpsimd.dma_start(dst, src)
                        gi += 1
                env["btrev"] = btrev

            def emit_expb():
                btrev = mlp_env["btrev"]
                expb = constp.tile([128, 512], BF16)
                for mb in range(2):
                    pe_ = auxpsum.tile([128, 512], F32, tag="aux2",
                                       name=f"pexpb{mb}")
                    nc.tensor.matmul(pe_[:, :256], jmat_r[:, :],
                                     btrev[1 - mb][:, :])
                    nc.scalar.activation(expb[:, 256 * mb:256 * mb + 256],
                                         pe_[:, :256], AF.Exp, scale=SCALE)
                return expb

            # --- main pipeline over 32 half-groups ---
            def emit_qk_exp(g, hh, pe_bias=False, split_exp=False):
                """QK matmuls + exp for half-group (g, hh) -> epair.

                pe_bias: accumulate sqrt(D)*bias into S via a J-matmul
                (needs btrev); exp then applies it, no DVE bias-mul needed.
                """
                ho = 512 * g + 256 * hh
                epair = []
                for half in range(2):
                    sp = spsum.tile([128, 1024], F32, tag="S",
                                    name=f"s{g}_{hh}_{half}")
                    for bi2 in range(2):
                        bi = 2 * half + bi2
                        fo = 512 * bi2
                        for mb in range(2):
                            out_ap = sp[:, fo + 256 * mb:fo + 256 * mb + 256]
                            nc.tensor.matmul(
                                out_ap,
                                k_all[32 * bi:32 * bi + 32,
                                      ho + 128 * mb:ho + 128 * mb + 128],
                                q_all[32 * bi:32 * bi + 32, ho:ho + 256],
                                tile_position=(32 * bi, 0),
                                start=True, stop=not pe_bias)
                            if pe_bias:
                                nc.tensor.matmul(
                                    out_ap, jmat_r[:, :],
                                    mlp_env["btrev"][1 - mb][:, :],
                                    tile_position=(0, 0),
                                    start=False, stop=True)
                    e = epool.tile([128, 1024], BF16, tag="E",
                                   name=f"e{g}_{hh}_{half}")
                    if split_exp:
                        # two 512-wide exps so the first can start as soon
                        # as the bi2=0 QK matmuls land (pipeline startup)
                        nc.scalar.activation(e[:, :512], sp[:, :512],
                                             AF.Exp, scale=SCALE)
                        nc.scalar.activation(e[:, 512:], sp[:, 512:],
                                             AF.Exp, scale=SCALE)
                    else:
                        nc.scalar.activation(e[:, :], sp[:, :], AF.Exp,
                                             scale=SCALE)
                    epair.append(e)
                return epair

            def emit_emul(expb, epair):
                for e in epair:
                    e4 = e[:, :].rearrange("p (j mb n) -> p j mb n",
                                           mb=2, n=256)
                    nc.vector.tensor_mul(
                        e4, e4,
                        expb[:, :].rearrange("p (mb n) -> p mb n", n=256)
                        .unsqueeze(1).broadcast_to((128, 2, 2, 256)))

            def emit_pv_out(g, hh, epair, split=False):
                o_ps = auxpsum.tile([128, 264], F32, tag="aux2",
                                    name=f"ops{g}_{hh}")

                def pv(bis):
                    for bi in bis:
                        e = epair[bi // 2]
                        fo = 512 * (bi % 2)
                        vb = 66 * (8 * g + 4 * hh + bi)
                        for nb in range(2):
                            j = 2 * bi + nb
                            for c in range(2):
                                nc.tensor.matmul(
                                    o_ps[:, 33 * j:33 * j + 33],
                                    e[:, fo + 256 * c + 128 * nb:
                                      fo + 256 * c + 128 * nb + 128],
                                    v_all[:, vb + 33 * c:vb + 33 * c + 33],
                                    start=(c == 0), stop=(c == 1))

                def norm_out(j0, nj):
                    rz = outp.tile([128, 8], F32, tag="rz",
                                   name=f"rz{g}_{hh}_{j0}")
                    o3 = o_ps[:, 33 * j0:33 * (j0 + nj)].rearrange(
                        "p (j e) -> p j e", e=33)
                    nc.vector.reciprocal(rz[:, :nj], o3[:, :, 32:33])
                    osb = outp.tile([128, 256], F32, tag="osb",
                                    name=f"osb{g}_{hh}_{j0}")
                    nc.vector.tensor_mul(
                        osb[:, :32 * nj].rearrange("p (j e) -> p j e", e=32),
                        o3[:, :, :32],
                        rz[:, :nj].unsqueeze(2).broadcast_to((128, nj, 32)))
                    nc.sync.dma_start(
                        AP(out_d, 256 * (2 * g + hh) + 32 * j0,
                           [[8192, 128], [1, 32 * nj]]),
                        osb[:, :32 * nj])

                if split:
                    # each j column-group of O (incl. its Z) is complete
                    # after its own matmuls: half the PV/norm/out overlaps
                    # the other half's exp at the kernel tail
                    pv((0, 1))
                    norm_out(0, 4)
                    pv((2, 3))
                    norm_out(4, 4)
                else:
                    pv(range(4))
                    norm_out(0, 8)

            # ---- schedule ----
            # unmul: exp'd, bias-mul not yet emitted; unpv: mul'd, PV not
            # yet emitted. Steady state: mul lags 1 half-group, PV lags 2,
            # and PVs are emitted BEFORE the new QK so their deps are
            # already satisfied when the PE sequencer reaches them.
            unmul = []
            unpv = []

            # Prefix: interleave the MLP's few PE ops between prefix QKs so
            # each lands in the in-order PE stream AFTER its DVE input is
            # ready but long BEFORE the prefix drains -> posd/gathers/expb
            # complete under the prefix exps.  MLP stage s goes after
            # half-group s+1 (first QK feeds Act ASAP, stage0 needs only
            # the tiny biasesT/wproj DMAs).
            stage_after = {1: 1, 2: 2, 3: 3, 4: 4}  # halfgroup -> mlp stage
            hg = 0
            emit_mlp_stage(0)
            for g in range(NPRE):
                ensure_chunk(CHUNK_OF_GROUP[min(g + 2, NGROUPS - 1)])
                for hh in range(2):
                    unmul.append((g, hh,
                                  emit_qk_exp(g, hh, split_exp=(g == 0)),
                                  False))
                    hg += 1
                    st = stage_after.get(hg)
                    if st == 4:
                        # keep Pool's queue ahead of the gathers
                        ensure_chunk(CHUNK_OF_GROUP[4])
                    if st is not None:
                        emit_mlp_stage(st)
            expb = emit_expb()

            FOLD = int(os.environ.get("K_FOLD", "2"))
            for g in range(NPRE, NGROUPS):
                ensure_chunk(CHUNK_OF_GROUP[min(g + 2, NGROUPS - 1)])
                for hh in range(2):
                    endgame = g >= NGROUPS - 3
                    drain = DRAIN if not endgame else 4
                    if endgame:
                        # lag 1 at the end: deps are long-satisfied and a
                        # short PE wait no longer starves anything
                        for _ in range(drain):
                            if unmul:
                                g_, hh_, ep_, folded = unmul.pop(0)
                                if not folded:
                                    emit_emul(expb, ep_)
                                unpv.append((g_, hh_, ep_))
                    for _ in range(drain):
                        if unpv:
                            emit_pv_out(*unpv.pop(0))
                    if not endgame:
                        for _ in range(drain):
                            if unmul:
                                g_, hh_, ep_, folded = unmul.pop(0)
                                if not folded:
                                    emit_emul(expb, ep_)
                                unpv.append((g_, hh_, ep_))
                    # g==NPRE stays unfolded: its exps need no btrev, so
                    # they bridge the gather-latency window right after
                    # the prefix; g==NPRE+1 folds both half-groups to
                    # compensate, keeping the total DVE mul count equal
                    fold = ((FOLD == 1)
                            or (FOLD == 2 and (2 * g + hh) % 2 == 0)
                            or g == NPRE + 1
                            or g >= NGROUPS - 2) and g != NPRE
                    unmul.append((g, hh, emit_qk_exp(g, hh, pe_bias=fold),
                                  fold))
            for g_, hh_, ep_, folded in unmul:
                if not folded:
                    emit_emul(expb, ep_)
                unpv.append((g_, hh_, ep_))
            # newest first: the critical exp->PV->norm->DMA chain of the
            # final half-group must not queue behind older pending norms
            for item in reversed(unpv):
                emit_pv_out(*item)

    nc.compile()
    _BUILD_CACHE["nc"] = nc
    return nc


def _host_constants():
    hh, ww = 16, 16
    bh, bw = np.meshgrid(np.arange(1 - hh, hh), np.arange(1 - ww, ww),
                         indexing="ij")
    biases = np.stack([bh, bw], -1).reshape(-1, 2).astype(np.float32)
    biasesT = np.zeros((2, 1024), np.float32)
    biasesT[:, :961] = biases.T
    return biasesT


def _blk8(w16):
    cout = w16.shape[1]
    blk = np.zeros((128, 8 * cout), np.float32)
    for j in range(8):
        blk[16 * j:16 * j + 16, cout * j:cout * j + cout] = w16
    return np.ascontiguousarray(blk)


def _tile16(vec):
    return np.ascontiguousarray(
        np.tile(np.asarray(vec, np.float32), (128, 8)))


def _tile16T(vec):
    # column-broadcast (transposed-space) tiling: out[16j+f, r] = vec[f]
    col = np.tile(np.asarray(vec, np.float32), 8)[:, None]
    return np.ascontiguousarray(np.tile(col, (1, 128)))


def build_in_maps(inputs):
    q = np.asarray(inputs["q"], np.float32)
    k = np.asarray(inputs["k"], np.float32)
    v = np.asarray(inputs["v"], np.float32)
    hh = int(np.asarray(inputs["h"]))
    ww = int(np.asarray(inputs["w"]))
    assert hh == 16 and ww == 16, (hh, ww)
    f32 = lambda name: np.asarray(inputs[name], np.float32)
    w3 = f32("w3")
    b3 = f32("b3")
    sqrtD = np.float32(np.sqrt(D))

    cblk = {
        "w1": _blk8(f32("w1")), "w2": _blk8(f32("w2")),
        "ident": np.eye(128, dtype=np.float32),
        "bprojt": _tile16(f32("b_proj")),
        "g1t": -_tile16(f32("ln1_g")), "lb1t": _tile16T(f32("ln1_b")),
        "linb1t": _tile16(f32("b1")),
        "g2t": -_tile16(f32("ln2_g")), "lb2t": _tile16T(f32("ln2_b")),
        "linb2t": _tile16(f32("b2")),
        "g3t": -_tile16(f32("ln3_g")), "lb3t": _tile16T(f32("ln3_b")),
    }
    shared = {
        "biasesT": _host_constants(),
        "wproj": f32("w_proj"),
        "jmat": np.eye(128, dtype=np.float32)[::-1].copy(),
    }

    def q_layout(x):
        # [128 w, 256 n, 32 d] -> [128 p=(bi,d), (g, hh, n)]
        x5 = x.reshape(16, 2, 4, 256, 32)           # g hh bi n d
        return np.ascontiguousarray(
            x5.transpose(2, 4, 0, 1, 3).reshape(128, 8192))

    def k_layout(x):
        # [128 w, 256 m, 32 d] -> [128 p=(bi,d), (g, hh, mb, m)]
        x6 = x.reshape(16, 2, 4, 2, 128, 32)        # g hh bi mb m d
        return np.ascontiguousarray(
            x6.transpose(2, 5, 0, 1, 3, 4).reshape(128, 8192))

    def pack_bf16_pairs(x):
        # f32 [128, 512] -> bf16 (RNE) bit-packed into uint32 [128, 256]
        u = np.ascontiguousarray(x).view(np.uint32)
        r = ((u + 0x7FFF + ((u >> 16) & 1)) >> 16).astype(np.uint32)
        return np.ascontiguousarray(r[:, 0::2] | (r[:, 1::2] << 16))

    def v_layout(x):
        # [128 p=m, (b 128, c 2, e 33)]; e==32 -> 1.0
        v4 = x.reshape(128, 2, 128, 32)             # b c p e
        out = np.ones((128, 128, 2, 33), np.float32)
        out[:, :, :, :32] = v4.transpose(2, 0, 1, 3)
        return np.ascontiguousarray(out.reshape(128, 8448))

    in_maps = []
    for c in range(NCORES):
        cbiga = np.empty((128, CONSTWA), np.float32)
        for nm, off in _CBA.items():
            cbiga[:, off:off + 128] = cblk[nm]
        cbigb = np.empty((128, CONSTWB), np.float32)
        for nm, off in _CBB.items():
            if nm == "w3c":
                cbigb[:, off:off + 8] = _blk8(w3[:, c:c + 1] * sqrtD)
            else:
                cbigb[:, off:off + 128] = cblk[nm]
        m = dict(shared)
        m["cbiga"] = np.ascontiguousarray(cbiga)
        m["cbigb"] = np.ascontiguousarray(cbigb)
        m["qd"] = q_layout(q[:, c])
        m["kd"] = k_layout(k[:, c])
        m["k0p"] = pack_bf16_pairs(m["kd"][:, 0:512])
        m["vd"] = v_layout(v[:, c])
        m["b3c"] = np.full((8, 1), b3[c], np.float32) * sqrtD
        in_maps.append(m)
    return in_maps


def unshard_out(raw):
    # raw [128 p, (g 16, hh 2, bi 4, nb 2, e 32)] -> [B, N, D]
    r6 = raw.reshape(128, 16, 2, 4, 2, 32)          # p g hh bi nb e
    return np.ascontiguousarray(
        r6.transpose(1, 2, 3, 4, 0, 5).reshape(128, 256, 32))


def kernel(**inputs):
    from concourse.bass_utils import run_bass_kernel_spmd

    nc = _build()
    in_maps = build_in_maps(inputs)
    res = run_bass_kernel_spmd(nc, in_maps, core_ids=list(range(NCORES)))
    out = np.empty((B, H, N, D), np.float32)
    for c in range(NCORES):
        out[:, c] = unshard_out(res.results[c]["out"])
    return out
